# revision 37
# baseline (speedup 1.0000x reference)
"""CapsuleLayer (dynamic routing) Trainium2 kernel — supertile rewrite.

Full inputs:  x (32, 2048, 32) f32, W (2048, 64, 32, 32) f32  [W indexed n,j,d,k]
Output:       v (32, 64, 32) f32

Sharding: input-capsule axis n split over 8 cores (256 each).

Per-core design (vs the per-group baseline):
  - 4 groups fused per "supertile" [128, 8192] fp16, free axis (gl, k, j)
    [k outer, j inner]; one DMA / one w-mult / one cu-mult per supertile
  - routing k-reduction runs ON THE TENSOR ENGINE: 32 accumulating identity
    matmuls (lhsT = I128) sum the k-slices of w = u*v_rep into PSUM, so the
    DVE never pays the 1x-mode tensor_reduce that dominated the baseline
  - b-update also on PE: for iter 2 an f32 identity matmul folds the old
    logits into the same PSUM accumulation group
  - softmax lives entirely on ACT: per-group exp with accum_out -> se,
    ln, nls = -ln(se)-10, then c = exp(b + nls) as a small [128, 256] tile
  - cu = c*u uses a stride-0-broadcast AP on c (k axis), keeping the DVE
    tensor_tensor in its 2x fp16 perf mode with no materialized c_rep
  - 4-stage pipeline DVE -> PE -> ACT -> DVE -> PE, software-pipelined with
    LAG=2 (cu(S) is emitted two supertiles behind w(S)) so every engine's
    in-order queue always has ready work
  - engine split: GPSIMD only does the early w block gl0; DVE does the other
    three w blocks (one strided op, v_rep broadcast along gl) + the cu sweep
  - W DMA'd per supertile (2 MB transfers, near peak BW); u_hat kept resident
    for R supertiles, rest spilled to DRAM fp16 and re-read once per iteration
  - iterations 1,2: fp16 128KB AllReduce of s over 8 cores, squash via
    s2*exp(-ln(1+s2)-0.5*ln(s2+eps)); v replicated to fp16 [128, (k j)]
  - iteration 3: each core ships its partial s3; host sums and squashes
  - single ACT table set (natural_log_exp_and_others) pinned via a Bacc
    subclass so the per-supertile Exp/Ln chain never reloads ACT tables
"""

import os
from contextlib import ExitStack

import numpy as np

B, NTOT, DD, J, K = 32, 2048, 32, 64, 32
JK = J * K
CORES = 8
NL = NTOT // CORES          # input capsules per core
ITERS = 3
F = 4                       # groups per supertile
FJK = F * JK

_CACHED = {}


def _build_nc(NL_, G_RES, n_cores, repeat=1):
    import concourse.bass as bass
    import concourse.mybir as mybir
    import concourse.tile as tile
    from concourse import bacc

    G = NL_ // 4            # groups of 4 input capsules
    SG = G // F             # supertiles
    R_SUP = min(max(1, G_RES // F), SG)   # resident supertiles
    NSPILL = SG - R_SUP
    f16 = mybir.dt.float16
    f32 = mybir.dt.float32
    AX = mybir.AxisListType
    OP = mybir.AluOpType
    AF = mybir.ActivationFunctionType

    import bass_rust as _bass_rust
    from concourse.hw_specs import get_activation_tables

    class _CapsBacc(bacc.Bacc):
        # Keep only a covering act-table set so the Exp/Ln/Copy chain never
        # reloads ACT tables (the stock pass thrashes sets).
        _ACT_KEEP = {"natural_log_exp_and_others", "sqrt_and_others"}

        def insert_act_table_loads(self):
            has_act = any(
                isinstance(i, mybir.InstActivation)
                for bb in self.main_func.blocks for i in bb.instructions
            )
            if not has_act:
                return
            tables = [
                (n, (f if n in self._ACT_KEEP else set()))
                for n, f in get_activation_tables(self.m.arch).items()
            ]
            _bass_rust.insert_act_table_loads(self, tables)

    nc = _CapsBacc()
    wd = nc.declare_dram_parameter("w", [SG, 128, FJK], f16, isOutput=False)
    xtd = nc.declare_dram_parameter("xt", [128, G * B], f16, isOutput=False)
    xbd_d = nc.declare_dram_parameter("xb", [128, G * 128], f16, isOutput=False)
    od_d = nc.declare_dram_parameter("od", [128, B], f16, isOutput=False)
    id_d = nc.declare_dram_parameter("id", [128, 128], f16, isOutput=False)
    idf_d = nc.declare_dram_parameter("idf", [128, 128], f32, isOutput=False)
    vd = nc.declare_dram_parameter("v", [B, JK], f32, isOutput=True)

    core_ids = list(range(n_cores))

    def bcast(ap_src, reps, pos):
        """Insert a stride-0 axis of length `reps` at free-dim position pos."""
        aps = [list(a) for a in ap_src.ap]
        aps.insert(1 + pos, [0, reps])
        return bass.AP(tensor=ap_src.tensor, offset=ap_src.offset, ap=aps)

    with tile.TileContext(nc) as tc, ExitStack() as ctx:
        const = ctx.enter_context(tc.tile_pool(name="const", bufs=1))
        dram = ctx.enter_context(tc.tile_pool(name="dram", bufs=1, space="DRAM"))
        ures = ctx.enter_context(tc.tile_pool(name="ures", bufs=1))
        stg = ctx.enter_context(tc.tile_pool(name="stg", bufs=5))  # half supertiles
        sm = ctx.enter_context(tc.tile_pool(name="small", bufs=1))
        smg = ctx.enter_context(tc.tile_pool(name="smallg", bufs=4))
        vrp = ctx.enter_context(tc.tile_pool(name="vrp", bufs=1))
        HJK = 2 * JK            # half-supertile free size

        # ---- constants ----
        odiag = const.tile([128, B], f16)   # odiag[p, b] = 1 if p % 32 == b
        nc.sync.dma_start(out=odiag, in_=od_d[:])
        id128 = const.tile([128, 128], f16)
        nc.sync.dma_start(out=id128, in_=id_d[:])
        idf128 = const.tile([128, 128], f32)
        nc.sync.dma_start(out=idf128, in_=idf_d[:])
        beps = const.tile([B, 1], f32)
        nc.vector.memset(beps, 1e-8)
        bm10 = const.tile([128, 1], f32)
        nc.vector.memset(bm10, -10.0)

        b_sb = const.tile([128, G * J], f32)    # routing logits per (n4 b)

        if NSPILL:
            u_spill = dram.tile([NSPILL, 128, FJK], f16)
        cc_in = dram.tile([B, JK], f16)
        cc_out = dram.tile([B, JK], f16)

        u_tiles = {}
        res_set = {s for s in range(SG) if (s * R_SUP) % SG < R_SUP}
        spill_idx = {}
        for s in range(SG):
            if s not in res_set:
                spill_idx[s] = len(spill_idx)

        def u_tile(s):
            if s in res_set:
                if s not in u_tiles:
                    u_tiles[s] = ures.tile(
                        [128, FJK], f16, tag=f"u{s}", name=f"u{s}"
                    )
                return u_tiles[s], True
            return None, False

        # ---------- squash + AllReduce of s; returns v_rep fp16 [128, (k j)] ----------
        def finish_iteration(s_psum, last):
            if last:
                s_sb = sm.tile([B, JK], f32, tag="s_work")
                nc.scalar.copy(s_sb, s_psum)
                # host gathers per-core partial s and finishes squash there
                nc.sync.dma_start(out=vd[:], in_=s_sb)
                return None
            s_sb16 = sm.tile([B, JK], f16, tag="s16")
            nc.scalar.copy(s_sb16, s_psum)
            nc.sync.dma_start(out=cc_in[:], in_=s_sb16)
            nc.gpsimd.collective_compute(
                "AllReduce",
                OP.add,
                ins=[cc_in[:].opt()],
                outs=[cc_out[:].opt()],
                replica_groups=[core_ids],
            )
            s16 = sm.tile([B, JK], f16, tag="s16", name="s_tot16")
            nc.sync.dma_start(out=s16, in_=cc_out[:])
            sq = sm.tile([B, JK], f16, tag="s_work", name="sq")
            nc.vector.tensor_mul(sq, s16, s16)
            # k-reduction tree (k outer, j inner): 2048 -> 64
            sqv = sq.rearrange("b (k j) -> b k j", k=K)
            q1 = sm.tile([B, 16, J], f16, tag="q1")
            nc.vector.tensor_add(q1, sqv[:, 0:16], sqv[:, 16:32])
            q2 = sm.tile([B, 8, J], f16, tag="q2")
            nc.vector.tensor_add(q2, q1[:, 0:8], q1[:, 8:16])
            q3 = sm.tile([B, 4, J], f16, tag="q3")
            nc.vector.tensor_add(q3, q2[:, 0:4], q2[:, 4:8])
            q4 = sm.tile([B, 2, J], f16, tag="q4")
            nc.vector.tensor_add(q4, q3[:, 0:2], q3[:, 2:4])
            s2 = sm.tile([B, 1, J], f32, tag="s2")
            nc.vector.tensor_add(s2, q4[:, 0:1], q4[:, 1:2])
            s2 = s2[:, 0]
            # squash scale: sc = s2/(1+s2)/sqrt(s2+eps)
            #             = s2 * exp(-ln(1+s2) - 0.5*ln(s2+eps))
            a_ln = sm.tile([B, J], f32, tag="a_ln")
            nc.scalar.activation(a_ln, s2, AF.Ln, bias=1.0, scale=1.0)
            b_ln = sm.tile([B, J], f32, tag="b_ln")
            nc.scalar.activation(b_ln, s2, AF.Ln, bias=beps, scale=1.0)
            comb = sm.tile([B, J], f32, tag="comb")
            nc.vector.scalar_tensor_tensor(
                comb, b_ln, -0.5, a_ln, op0=OP.mult, op1=OP.subtract
            )
            e_sc = sm.tile([B, J], f32, tag="e_sc")
            nc.scalar.activation(e_sc, comb, AF.Exp)
            sc16 = sm.tile([B, 1, J], f16, tag="sc16")
            nc.vector.tensor_mul(sc16[:, 0], s2, e_sc)
            # replicate sc along k (k outer => contiguous doubling)
            scr = sm.tile([B, K, J], f16, tag="scr")
            nc.vector.tensor_copy(scr[:, 0:1], sc16)
            nc.vector.tensor_copy(scr[:, 1:2], scr[:, 0:1])
            nc.vector.tensor_copy(scr[:, 2:4], scr[:, 0:2])
            nc.vector.tensor_copy(scr[:, 4:8], scr[:, 0:4])
            nc.vector.tensor_copy(scr[:, 8:16], scr[:, 0:8])
            nc.vector.tensor_copy(scr[:, 16:32], scr[:, 0:16])
            v16 = sm.tile([B, JK], f16, tag="s_work", name="v16")
            nc.vector.tensor_mul(v16, s16, scr.rearrange("b k j -> b (k j)"))
            v_rep = vrp.tile([128, JK], f16, tag="v_rep")
            for r in range(4):
                rs = slice(32 * r, 32 * r + 32)
                nc.vector.tensor_copy(v_rep[rs, :], v16)
            return v_rep

        # ================= pass 1: u_hat + s1 =================
        for rep in range(repeat):
          with tc.tile_pool(name=f"wp{rep}", bufs=2) as wp, \
             tc.tile_pool(name=f"p1c{rep}", bufs=1) as p1c, \
             tc.tile_pool(name=f"pu{rep}", bufs=2, space="PSUM") as pu, \
             tc.tile_pool(name=f"ps1{rep}", bufs=1, space="PSUM") as ps1:
              xts = p1c.tile([128, G * B], f16)
              nc.sync.dma_start(out=xts, in_=xtd[:])
              xbd = p1c.tile([128, G * 128], f16)   # block-diag x per group
              nc.sync.dma_start(out=xbd, in_=xbd_d[:])

              s1_psum = ps1.tile([B, JK], f32)
              for S in range(SG):
                  wt = wp.tile([128, FJK], f16, tag="wt")
                  nc.sync.dma_start(out=wt, in_=wd[S])
                  ut, resident = u_tile(S)
                  uth = None
                  for gl in range(F):
                      g = S * F + gl
                      if not resident and gl % 2 == 0:
                          uth = stg.tile([128, HJK], f16, tag="ustg")
                      xbsl = xbd[:, g * 128:(g + 1) * 128]
                      xsl = xts[:, g * B:(g + 1) * B]
                      for h in range(2):
                          up = pu.tile([128, 1024], f32, tag="up")
                          for cch in range(2):
                              fl = gl * JK + h * 1024 + cch * 512
                              psl = slice(cch * 512, cch * 512 + 512)
                              nc.tensor.matmul(
                                  up[:, psl],
                                  lhsT=xbsl,
                                  rhs=wt[:, fl:fl + 512],
                                  start=True, stop=True,
                                  skip_group_check=True,
                              )
                              sl = slice(h * 1024 + cch * 512,
                                         h * 1024 + cch * 512 + 512)
                              nc.tensor.matmul(
                                  s1_psum[:, sl],
                                  lhsT=xsl,
                                  rhs=wt[:, fl:fl + 512],
                                  start=(g == 0), stop=(g == G - 1),
                                  skip_group_check=True,
                              )
                          if resident:
                              dst = ut[:, gl * JK + h * 1024:
                                       gl * JK + (h + 1) * 1024]
                          else:
                              dst = uth[:, (gl % 2) * JK + h * 1024:
                                        (gl % 2) * JK + (h + 1) * 1024]
                          if (gl + h) % 2 == 0:
                              nc.vector.tensor_copy(dst, up)
                          else:
                              nc.scalar.copy(dst, up)
                      if not resident and gl % 2 == 1:
                          hh = gl // 2
                          nc.scalar.dma_start(
                              out=u_spill[spill_idx[S]][:, hh * HJK:
                                                        (hh + 1) * HJK],
                              in_=uth,
                          )
              v_rep = finish_iteration(s1_psum, last=False)

          # ================= passes 2..ITERS =================
          with tc.tile_pool(name=f"big{rep}", bufs=4) as big1, \
             tc.tile_pool(name=f"pt{rep}", bufs=2, space="PSUM") as ptp, \
             tc.tile_pool(name=f"ps23{rep}", bufs=1, space="PSUM") as ps23:
              LAG = 2   # software pipeline: DVE runs w(S+1..S+LAG) while
                        # PE->ACT produce c(S); cu(S) then follows on DVE
              for it in range(1, ITERS):
                  s_psum = ps23.tile([B, JK], f32, tag="s23")
                  state = {}

                  def front(S):
                      ut, resident = u_tile(S)
                      if resident:
                          uhalf = [ut[:, 0:HJK], ut[:, HJK:FJK]]
                      else:
                          uhalf = []
                          for hh in range(2):
                              uh = stg.tile([128, HJK], f16, tag="ustg")
                              nc.sync.dma_start(
                                  out=uh,
                                  in_=u_spill[spill_idx[S]][:, hh * HJK:
                                                            (hh + 1) * HJK],
                              )
                              uhalf.append(uh)

                      def usl(gl):
                          return uhalf[gl // 2][:, (gl % 2) * JK:
                                                (gl % 2 + 1) * JK]

                      wcu = big1.tile([128, FJK], f16, tag="wcu")
                      wv = wcu.rearrange("p (gl k j) -> p gl k j", gl=F, k=K)
                      # w = u * v_rep; GPSIMD takes block gl0 (early, off the
                      # critical path), DVE the other three in one strided op
                      # with v_rep broadcast along gl (stride-0 outer axis)
                      nc.gpsimd.tensor_mul(wcu[:, 0:JK], usl(0), v_rep)
                      vr3 = bcast(v_rep.rearrange("p (k j) -> p k j", k=K),
                                  F - 1, 0)
                      if resident:
                          nc.vector.tensor_mul(
                              wv[:, 1:F],
                              ut.rearrange("p (gl k j) -> p gl k j",
                                           gl=F, k=K)[:, 1:F],
                              vr3,
                          )
                      else:
                          vr1 = bcast(
                              v_rep.rearrange("p (k j) -> p k j", k=K), 1, 0)
                          vr2 = bcast(
                              v_rep.rearrange("p (k j) -> p k j", k=K), 2, 0)
                          uv0 = uhalf[0].rearrange(
                              "p (gh k j) -> p gh k j", gh=2, k=K)
                          uv1 = uhalf[1].rearrange(
                              "p (gh k j) -> p gh k j", gh=2, k=K)
                          nc.vector.tensor_mul(wv[:, 1:2], uv0[:, 1:2], vr1)
                          nc.vector.tensor_mul(wv[:, 2:4], uv1, vr2)
                      # k-reduction on PE: t[p, (gl j)] = sum_k w[p,(gl k j)]
                      # two identity matmuls: k=0 resets PSUM; k=1..31 stream
                      # 7936 cols whose out AP cycles the same 256 PSUM words
                      # (stride-0 k axis) so each revisit accumulates
                      t_psum = ptp.tile([128, F * J], f32, tag="tp")
                      tv = t_psum.rearrange("p (gl j) -> p gl j", gl=F)
                      bsl = b_sb[:, S * F * J:(S + 1) * F * J]
                      for k in range(K):
                          nc.tensor.matmul(
                              tv,
                              lhsT=id128,
                              rhs=wv[:, :, k],
                              start=(k == 0), stop=(k == K - 1 and it == 1),
                              skip_group_check=True,
                          )
                      if it > 1:
                          # fold the old logits in on PE (f32 identity matmul)
                          nc.tensor.matmul(
                              t_psum,
                              lhsT=idf128,
                              rhs=bsl,
                              start=False, stop=True,
                              skip_group_check=True,
                          )
                      # b writeback + softmax, entirely on ACT:
                      #   b = t_psum; e_g = exp(b_g - 10) with accum -> se_g;
                      #   nls_g = -ln(se_g) - 10; c_g = exp(b_g + nls_g)
                      nc.scalar.copy(bsl, t_psum)
                      se = smg.tile([128, F], f32, tag="se")
                      e16 = smg.tile([128, F * J], f16, tag="e16")
                      for gl in range(F):
                          nc.scalar.activation(
                              e16[:, gl * J:(gl + 1) * J],
                              bsl[:, gl * J:(gl + 1) * J],
                              AF.Exp, bias=bm10, scale=1.0,
                              accum_out=se[:, gl:gl + 1],
                          )
                      ls = smg.tile([128, F], f32, tag="ls")
                      nc.scalar.activation(ls, se, AF.Ln)
                      nls = smg.tile([128, F], f32, tag="nls")
                      nc.scalar.activation(nls, ls, AF.Copy, bias=-10.0,
                                           scale=-1.0)
                      c16 = smg.tile([128, F * J], f16, tag="c16")
                      for gl in range(F):
                          nc.scalar.activation(
                              c16[:, gl * J:(gl + 1) * J],
                              bsl[:, gl * J:(gl + 1) * J],
                              AF.Exp, bias=nls[:, gl:gl + 1], scale=1.0,
                          )
                      return dict(resident=resident, ut=ut, uhalf=uhalf,
                                  c16=c16)

                  def back(S, st):
                      resident, ut, uhalf, c16 = (st["resident"], st["ut"],
                                                  st["uhalf"], st["c16"])
                      # cu = c * u with c broadcast along k (stride-0 axis)
                      cu = big1.tile([128, FJK], f16, tag="wcu", name="cu")
                      cuv = cu.rearrange("p (gl k j) -> p gl k j", gl=F, k=K)
                      c4 = c16.rearrange("p (gl j) -> p gl j", gl=F)
                      if resident:
                          nc.vector.tensor_mul(
                              cuv,
                              ut.rearrange("p (gl k j) -> p gl k j",
                                           gl=F, k=K),
                              bcast(c4, K, 1),
                          )
                      else:
                          uv0 = uhalf[0].rearrange(
                              "p (gh k j) -> p gh k j", gh=2, k=K)
                          uv1 = uhalf[1].rearrange(
                              "p (gh k j) -> p gh k j", gh=2, k=K)
                          nc.vector.tensor_mul(
                              cuv[:, 0:2], uv0, bcast(c4[:, 0:2], K, 1))
                          nc.vector.tensor_mul(
                              cuv[:, 2:4], uv1, bcast(c4[:, 2:4], K, 1))
                      for gl in range(F):
                          for cch in range(4):
                              sl = slice(cch * 512, cch * 512 + 512)
                              nc.tensor.matmul(
                                  s_psum[:, sl],
                                  lhsT=odiag,
                                  rhs=cu[:, gl * JK + cch * 512:
                                         gl * JK + cch * 512 + 512],
                                  start=(S == 0 and gl == 0),
                                  stop=(S == SG - 1 and gl == F - 1),
                                  skip_group_check=True,
                              )

                  for S in range(SG + LAG):
                      if S < SG:
                          state[S] = front(S)
                      if S >= LAG:
                          back(S - LAG, state.pop(S - LAG))
                  v_rep = finish_iteration(s_psum, last=(it == ITERS - 1))

    nc.finalize()
    return nc


def _pack_inputs(x, W, n_cores, ntot=NTOT):
    """Shard over n, cast fp16, pre-transpose to the on-chip layouts."""
    nl = ntot // n_cores
    g = nl // 4
    sg = g // F
    in_maps = []
    for c in range(n_cores):
        wl = W[c * nl:(c + 1) * nl]                       # (nl, J, D, K)
        # supertile layout: [S, (n4 d), (gl, k, j)]
        wp = wl.reshape(sg, F, 4, J, DD, K)               # S gl n4 j d k
        wp = wp.transpose(0, 2, 4, 1, 5, 3)               # S n4 d gl k j
        wp = np.ascontiguousarray(
            wp.reshape(sg, 128, F * K * J).astype(np.float16)
        )
        xl = x[:, c * nl:(c + 1) * nl, :]                 # (B, nl, D)
        xg = xl.transpose(1, 2, 0).reshape(g, 4, DD, B).astype(np.float16)
        xt = np.ascontiguousarray(
            xg.reshape(g, 128, B).transpose(1, 0, 2)      # (128, g, b)
            .reshape(128, g * B)
        ) / np.float16(J)                                 # fold s1's 1/J scale
        xt = xt.astype(np.float16)
        xb = np.zeros((g, 128, 128), np.float16)
        for ns in range(4):
            xb[:, ns * 32:(ns + 1) * 32, ns * 32:(ns + 1) * 32] = xg[:, ns]
        xb = np.ascontiguousarray(
            xb.transpose(1, 0, 2).reshape(128, g * 128)
        )
        od = np.tile(np.eye(32, dtype=np.float16), (4, 1))
        in_maps.append({"w": wp, "xt": xt, "xb": xb, "od": od,
                        "id": np.eye(128, dtype=np.float16),
                        "idf": np.eye(128, dtype=np.float32)})
    return in_maps


def kernel(x, W):
    from concourse.bass_utils import run_bass_kernel_spmd

    x = np.asarray(x, dtype=np.float32)
    W = np.asarray(W, dtype=np.float32)
    g_res = int(os.environ.get("CAPS_G_RES", "12"))
    key = (NL, g_res, CORES)
    if key not in _CACHED:
        _CACHED[key] = _build_nc(NL, g_res, CORES)
    nc = _CACHED[key]
    in_maps = _pack_inputs(x, W, CORES)
    res = run_bass_kernel_spmd(nc, in_maps, list(range(CORES)))
    s = np.zeros((B, JK), np.float32)
    for c in range(CORES):
        s += np.asarray(res.results[c]["v"], dtype=np.float32)
    s = s.reshape(B, K, J).transpose(0, 2, 1)             # (k j) -> (B, J, K)
    s = np.ascontiguousarray(s)
    s2 = np.sum(s * s, axis=-1, keepdims=True)
    v = s2 / (1.0 + s2) / np.sqrt(s2 + 1e-8) * s
    return v.astype(np.float32)


# revision 38
# speedup vs baseline: 1.0079x; 1.0079x over previous
"""CapsuleLayer (dynamic routing) Trainium2 kernel — supertile rewrite.

Full inputs:  x (32, 2048, 32) f32, W (2048, 64, 32, 32) f32  [W indexed n,j,d,k]
Output:       v (32, 64, 32) f32

Sharding: input-capsule axis n split over 8 cores (256 each).

Per-core design (vs the per-group baseline):
  - 4 groups fused per "supertile" [128, 8192] fp16, free axis (gl, k, j)
    [k outer, j inner]; one DMA / one w-mult / one cu-mult per supertile
  - routing k-reduction runs ON THE TENSOR ENGINE: 32 accumulating identity
    matmuls (lhsT = I128) sum the k-slices of w = u*v_rep into PSUM, so the
    DVE never pays the 1x-mode tensor_reduce that dominated the baseline
  - b-update also on PE: for iter 2 an f32 identity matmul folds the old
    logits into the same PSUM accumulation group
  - softmax lives entirely on ACT: per-group exp with accum_out -> se,
    ln, nls = -ln(se)-10, then c = exp(b + nls) as a small [128, 256] tile
  - cu = c*u uses a stride-0-broadcast AP on c (k axis), keeping the DVE
    tensor_tensor in its 2x fp16 perf mode with no materialized c_rep
  - 4-stage pipeline DVE -> PE -> ACT -> DVE -> PE, software-pipelined with
    LAG=2 (cu(S) is emitted two supertiles behind w(S)) so every engine's
    in-order queue always has ready work
  - engine split: GPSIMD only does the early w block gl0; DVE does the other
    three w blocks (one strided op, v_rep broadcast along gl) + the cu sweep
  - W DMA'd per supertile (2 MB transfers, near peak BW); u_hat kept resident
    for R supertiles, rest spilled to DRAM fp16 and re-read once per iteration
  - iterations 1,2: fp16 128KB AllReduce of s over 8 cores, squash via
    s2*exp(-ln(1+s2)-0.5*ln(s2+eps)); v replicated to fp16 [128, (k j)]
  - iteration 3: each core ships its partial s3; host sums and squashes
  - single ACT table set (natural_log_exp_and_others) pinned via a Bacc
    subclass so the per-supertile Exp/Ln chain never reloads ACT tables
"""

import os
from contextlib import ExitStack

import numpy as np

B, NTOT, DD, J, K = 32, 2048, 32, 64, 32
JK = J * K
CORES = 8
NL = NTOT // CORES          # input capsules per core
ITERS = 3
F = 4                       # groups per supertile
FJK = F * JK

_CACHED = {}


def _build_nc(NL_, G_RES, n_cores, repeat=1):
    import concourse.bass as bass
    import concourse.mybir as mybir
    import concourse.tile as tile
    from concourse import bacc

    G = NL_ // 4            # groups of 4 input capsules
    SG = G // F             # supertiles
    R_SUP = min(max(1, G_RES // F), SG)   # resident supertiles
    NSPILL = SG - R_SUP
    f16 = mybir.dt.float16
    f32 = mybir.dt.float32
    AX = mybir.AxisListType
    OP = mybir.AluOpType
    AF = mybir.ActivationFunctionType

    import bass_rust as _bass_rust
    from concourse.hw_specs import get_activation_tables

    class _CapsBacc(bacc.Bacc):
        # Keep only a covering act-table set so the Exp/Ln/Copy chain never
        # reloads ACT tables (the stock pass thrashes sets).
        _ACT_KEEP = {"natural_log_exp_and_others", "sqrt_and_others"}

        def insert_act_table_loads(self):
            has_act = any(
                isinstance(i, mybir.InstActivation)
                for bb in self.main_func.blocks for i in bb.instructions
            )
            if not has_act:
                return
            tables = [
                (n, (f if n in self._ACT_KEEP else set()))
                for n, f in get_activation_tables(self.m.arch).items()
            ]
            _bass_rust.insert_act_table_loads(self, tables)

    nc = _CapsBacc()
    wd = nc.declare_dram_parameter("w", [SG, 128, FJK], f16, isOutput=False)
    xtd = nc.declare_dram_parameter("xt", [128, G * B], f16, isOutput=False)
    xbd_d = nc.declare_dram_parameter("xb", [128, G * 128], f16, isOutput=False)
    od_d = nc.declare_dram_parameter("od", [128, B], f16, isOutput=False)
    id_d = nc.declare_dram_parameter("id", [128, 128], f16, isOutput=False)
    idf_d = nc.declare_dram_parameter("idf", [128, 128], f32, isOutput=False)
    vd = nc.declare_dram_parameter("v", [B, JK], f32, isOutput=True)

    core_ids = list(range(n_cores))

    def bcast(ap_src, reps, pos):
        """Insert a stride-0 axis of length `reps` at free-dim position pos."""
        aps = [list(a) for a in ap_src.ap]
        aps.insert(1 + pos, [0, reps])
        return bass.AP(tensor=ap_src.tensor, offset=ap_src.offset, ap=aps)

    with tile.TileContext(nc) as tc, ExitStack() as ctx:
        const = ctx.enter_context(tc.tile_pool(name="const", bufs=1))
        dram = ctx.enter_context(tc.tile_pool(name="dram", bufs=1, space="DRAM"))
        ures = ctx.enter_context(tc.tile_pool(name="ures", bufs=1))
        stg = ctx.enter_context(tc.tile_pool(name="stg", bufs=5))  # half supertiles
        sm = ctx.enter_context(tc.tile_pool(name="small", bufs=1))
        smg = ctx.enter_context(tc.tile_pool(name="smallg", bufs=4))
        vrp = ctx.enter_context(tc.tile_pool(name="vrp", bufs=1))
        HJK = 2 * JK            # half-supertile free size

        # ---- constants ----
        odiag = const.tile([128, B], f16)   # odiag[p, b] = 1 if p % 32 == b
        nc.sync.dma_start(out=odiag, in_=od_d[:])
        id128 = const.tile([128, 128], f16)
        nc.sync.dma_start(out=id128, in_=id_d[:])
        idf128 = const.tile([128, 128], f32)
        nc.sync.dma_start(out=idf128, in_=idf_d[:])
        beps = const.tile([B, 1], f32)
        nc.vector.memset(beps, 1e-8)
        bm10 = const.tile([128, 1], f32)
        nc.vector.memset(bm10, -10.0)

        b_sb = const.tile([128, G * J], f32)    # routing logits per (n4 b)

        if NSPILL:
            u_spill = dram.tile([NSPILL, 128, FJK], f16)
        cc_in = dram.tile([B, JK], f16)
        cc_out = dram.tile([B, JK], f16)

        u_tiles = {}
        res_set = {s for s in range(SG) if (s * R_SUP) % SG < R_SUP}
        spill_idx = {}
        for s in range(SG):
            if s not in res_set:
                spill_idx[s] = len(spill_idx)

        def u_tile(s):
            if s in res_set:
                if s not in u_tiles:
                    u_tiles[s] = ures.tile(
                        [128, FJK], f16, tag=f"u{s}", name=f"u{s}"
                    )
                return u_tiles[s], True
            return None, False

        # ---------- squash + AllReduce of s; returns v_rep fp16 [128, (k j)] ----------
        def finish_iteration(s_psum, last):
            if last:
                s_sb = sm.tile([B, JK], f32, tag="s_work")
                nc.scalar.copy(s_sb, s_psum)
                # host gathers per-core partial s and finishes squash there
                nc.sync.dma_start(out=vd[:], in_=s_sb)
                return None
            s_sb16 = sm.tile([B, JK], f16, tag="s16")
            nc.scalar.copy(s_sb16, s_psum)
            nc.scalar.dma_start(out=cc_in[:], in_=s_sb16)
            nc.gpsimd.collective_compute(
                "AllReduce",
                OP.add,
                ins=[cc_in[:].opt()],
                outs=[cc_out[:].opt()],
                replica_groups=[core_ids],
            )
            s16 = sm.tile([B, JK], f16, tag="s16", name="s_tot16")
            nc.scalar.dma_start(out=s16, in_=cc_out[:])
            sq = sm.tile([B, JK], f16, tag="s_work", name="sq")
            nc.vector.tensor_mul(sq, s16, s16)
            # k-reduction tree (k outer, j inner): 2048 -> 64
            sqv = sq.rearrange("b (k j) -> b k j", k=K)
            q1 = sm.tile([B, 16, J], f16, tag="q1")
            nc.vector.tensor_add(q1, sqv[:, 0:16], sqv[:, 16:32])
            q2 = sm.tile([B, 8, J], f16, tag="q2")
            nc.vector.tensor_add(q2, q1[:, 0:8], q1[:, 8:16])
            q3 = sm.tile([B, 4, J], f16, tag="q3")
            nc.vector.tensor_add(q3, q2[:, 0:4], q2[:, 4:8])
            q4 = sm.tile([B, 2, J], f16, tag="q4")
            nc.vector.tensor_add(q4, q3[:, 0:2], q3[:, 2:4])
            s2 = sm.tile([B, 1, J], f32, tag="s2")
            nc.vector.tensor_add(s2, q4[:, 0:1], q4[:, 1:2])
            s2 = s2[:, 0]
            # squash scale: sc = s2/(1+s2)/sqrt(s2+eps)
            #             = s2 * exp(-ln(1+s2) - 0.5*ln(s2+eps))
            a_ln = sm.tile([B, J], f32, tag="a_ln")
            nc.scalar.activation(a_ln, s2, AF.Ln, bias=1.0, scale=1.0)
            b_ln = sm.tile([B, J], f32, tag="b_ln")
            nc.scalar.activation(b_ln, s2, AF.Ln, bias=beps, scale=1.0)
            comb = sm.tile([B, J], f32, tag="comb")
            nc.vector.scalar_tensor_tensor(
                comb, b_ln, -0.5, a_ln, op0=OP.mult, op1=OP.subtract
            )
            e_sc = sm.tile([B, J], f32, tag="e_sc")
            nc.scalar.activation(e_sc, comb, AF.Exp)
            sc16 = sm.tile([B, 1, J], f16, tag="sc16")
            nc.vector.tensor_mul(sc16[:, 0], s2, e_sc)
            # replicate sc along k (k outer => contiguous doubling)
            scr = sm.tile([B, K, J], f16, tag="scr")
            nc.vector.tensor_copy(scr[:, 0:1], sc16)
            nc.vector.tensor_copy(scr[:, 1:2], scr[:, 0:1])
            nc.vector.tensor_copy(scr[:, 2:4], scr[:, 0:2])
            nc.vector.tensor_copy(scr[:, 4:8], scr[:, 0:4])
            nc.vector.tensor_copy(scr[:, 8:16], scr[:, 0:8])
            nc.vector.tensor_copy(scr[:, 16:32], scr[:, 0:16])
            v16 = sm.tile([B, JK], f16, tag="s_work", name="v16")
            nc.vector.tensor_mul(v16, s16, scr.rearrange("b k j -> b (k j)"))
            v_rep = vrp.tile([128, JK], f16, tag="v_rep")
            for r in range(4):
                rs = slice(32 * r, 32 * r + 32)
                nc.vector.tensor_copy(v_rep[rs, :], v16)
            return v_rep

        # ================= pass 1: u_hat + s1 =================
        for rep in range(repeat):
          with tc.tile_pool(name=f"wp{rep}", bufs=2) as wp, \
             tc.tile_pool(name=f"p1c{rep}", bufs=1) as p1c, \
             tc.tile_pool(name=f"pu{rep}", bufs=2, space="PSUM") as pu, \
             tc.tile_pool(name=f"ps1{rep}", bufs=1, space="PSUM") as ps1:
              xts = p1c.tile([128, G * B], f16)
              nc.sync.dma_start(out=xts, in_=xtd[:])
              xbd = p1c.tile([128, G * 128], f16)   # block-diag x per group
              nc.sync.dma_start(out=xbd, in_=xbd_d[:])

              s1_psum = ps1.tile([B, JK], f32)
              for S in range(SG):
                  wt = wp.tile([128, FJK], f16, tag="wt")
                  nc.sync.dma_start(out=wt, in_=wd[S])
                  ut, resident = u_tile(S)
                  uth = None
                  for gl in range(F):
                      g = S * F + gl
                      if not resident and gl % 2 == 0:
                          uth = stg.tile([128, HJK], f16, tag="ustg")
                      xbsl = xbd[:, g * 128:(g + 1) * 128]
                      xsl = xts[:, g * B:(g + 1) * B]
                      for h in range(2):
                          up = pu.tile([128, 1024], f32, tag="up")
                          for cch in range(2):
                              fl = gl * JK + h * 1024 + cch * 512
                              psl = slice(cch * 512, cch * 512 + 512)
                              nc.tensor.matmul(
                                  up[:, psl],
                                  lhsT=xbsl,
                                  rhs=wt[:, fl:fl + 512],
                                  start=True, stop=True,
                                  skip_group_check=True,
                              )
                          for cch in range(2):
                              fl = gl * JK + h * 1024 + cch * 512
                              sl = slice(h * 1024 + cch * 512,
                                         h * 1024 + cch * 512 + 512)
                              nc.tensor.matmul(
                                  s1_psum[:, sl],
                                  lhsT=xsl,
                                  rhs=wt[:, fl:fl + 512],
                                  start=(g == 0), stop=(g == G - 1),
                                  skip_group_check=True,
                              )
                          if resident:
                              dst = ut[:, gl * JK + h * 1024:
                                       gl * JK + (h + 1) * 1024]
                          else:
                              dst = uth[:, (gl % 2) * JK + h * 1024:
                                        (gl % 2) * JK + (h + 1) * 1024]
                          if (gl + h) % 2 == 0:
                              nc.vector.tensor_copy(dst, up)
                          else:
                              nc.scalar.copy(dst, up)
                      if not resident and gl % 2 == 1:
                          hh = gl // 2
                          nc.scalar.dma_start(
                              out=u_spill[spill_idx[S]][:, hh * HJK:
                                                        (hh + 1) * HJK],
                              in_=uth,
                          )
              v_rep = finish_iteration(s1_psum, last=False)

          # ================= passes 2..ITERS =================
          with tc.tile_pool(name=f"big{rep}", bufs=4) as big1, \
             tc.tile_pool(name=f"pt{rep}", bufs=2, space="PSUM") as ptp, \
             tc.tile_pool(name=f"ps23{rep}", bufs=1, space="PSUM") as ps23:
              LAG = 2   # software pipeline: DVE runs w(S+1..S+LAG) while
                        # PE->ACT produce c(S); cu(S) then follows on DVE
              for it in range(1, ITERS):
                  s_psum = ps23.tile([B, JK], f32, tag="s23")
                  state = {}

                  def front(S):
                      ut, resident = u_tile(S)
                      if resident:
                          uhalf = [ut[:, 0:HJK], ut[:, HJK:FJK]]
                      else:
                          uhalf = []
                          for hh in range(2):
                              uh = stg.tile([128, HJK], f16, tag="ustg")
                              nc.sync.dma_start(
                                  out=uh,
                                  in_=u_spill[spill_idx[S]][:, hh * HJK:
                                                            (hh + 1) * HJK],
                              )
                              uhalf.append(uh)

                      def usl(gl):
                          return uhalf[gl // 2][:, (gl % 2) * JK:
                                                (gl % 2 + 1) * JK]

                      wcu = big1.tile([128, FJK], f16, tag="wcu")
                      wv = wcu.rearrange("p (gl k j) -> p gl k j", gl=F, k=K)
                      # w = u * v_rep; GPSIMD takes block gl0 (early, off the
                      # critical path), DVE the other three in one strided op
                      # with v_rep broadcast along gl (stride-0 outer axis)
                      nc.gpsimd.tensor_mul(wcu[:, 0:JK], usl(0), v_rep)
                      vr3 = bcast(v_rep.rearrange("p (k j) -> p k j", k=K),
                                  F - 1, 0)
                      if resident:
                          nc.vector.tensor_mul(
                              wv[:, 1:F],
                              ut.rearrange("p (gl k j) -> p gl k j",
                                           gl=F, k=K)[:, 1:F],
                              vr3,
                          )
                      else:
                          vr1 = bcast(
                              v_rep.rearrange("p (k j) -> p k j", k=K), 1, 0)
                          vr2 = bcast(
                              v_rep.rearrange("p (k j) -> p k j", k=K), 2, 0)
                          uv0 = uhalf[0].rearrange(
                              "p (gh k j) -> p gh k j", gh=2, k=K)
                          uv1 = uhalf[1].rearrange(
                              "p (gh k j) -> p gh k j", gh=2, k=K)
                          nc.vector.tensor_mul(wv[:, 1:2], uv0[:, 1:2], vr1)
                          nc.vector.tensor_mul(wv[:, 2:4], uv1, vr2)
                      # k-reduction on PE: t[p, (gl j)] = sum_k w[p,(gl k j)]
                      # two identity matmuls: k=0 resets PSUM; k=1..31 stream
                      # 7936 cols whose out AP cycles the same 256 PSUM words
                      # (stride-0 k axis) so each revisit accumulates
                      t_psum = ptp.tile([128, F * J], f32, tag="tp")
                      tv = t_psum.rearrange("p (gl j) -> p gl j", gl=F)
                      bsl = b_sb[:, S * F * J:(S + 1) * F * J]
                      for k in range(K):
                          nc.tensor.matmul(
                              tv,
                              lhsT=id128,
                              rhs=wv[:, :, k],
                              start=(k == 0), stop=(k == K - 1 and it == 1),
                              skip_group_check=True,
                          )
                      if it > 1:
                          # fold the old logits in on PE (f32 identity matmul)
                          nc.tensor.matmul(
                              t_psum,
                              lhsT=idf128,
                              rhs=bsl,
                              start=False, stop=True,
                              skip_group_check=True,
                          )
                      # b writeback + softmax, entirely on ACT:
                      #   b = t_psum; e_g = exp(b_g - 10) with accum -> se_g;
                      #   nls_g = -ln(se_g) - 10; c_g = exp(b_g + nls_g)
                      nc.scalar.copy(bsl, t_psum)
                      se = smg.tile([128, F], f32, tag="se")
                      e16 = smg.tile([128, F * J], f16, tag="e16")
                      for gl in range(F):
                          nc.scalar.activation(
                              e16[:, gl * J:(gl + 1) * J],
                              bsl[:, gl * J:(gl + 1) * J],
                              AF.Exp, bias=bm10, scale=1.0,
                              accum_out=se[:, gl:gl + 1],
                          )
                      ls = smg.tile([128, F], f32, tag="ls")
                      nc.scalar.activation(ls, se, AF.Ln)
                      nls = smg.tile([128, F], f32, tag="nls")
                      nc.scalar.activation(nls, ls, AF.Copy, bias=-10.0,
                                           scale=-1.0)
                      c16 = smg.tile([128, F * J], f16, tag="c16")
                      for gl in range(F):
                          nc.scalar.activation(
                              c16[:, gl * J:(gl + 1) * J],
                              bsl[:, gl * J:(gl + 1) * J],
                              AF.Exp, bias=nls[:, gl:gl + 1], scale=1.0,
                          )
                      return dict(resident=resident, ut=ut, uhalf=uhalf,
                                  c16=c16)

                  def back(S, st):
                      resident, ut, uhalf, c16 = (st["resident"], st["ut"],
                                                  st["uhalf"], st["c16"])
                      # cu = c * u with c broadcast along k (stride-0 axis)
                      cu = big1.tile([128, FJK], f16, tag="wcu", name="cu")
                      cuv = cu.rearrange("p (gl k j) -> p gl k j", gl=F, k=K)
                      c4 = c16.rearrange("p (gl j) -> p gl j", gl=F)
                      if resident:
                          nc.vector.tensor_mul(
                              cuv,
                              ut.rearrange("p (gl k j) -> p gl k j",
                                           gl=F, k=K),
                              bcast(c4, K, 1),
                          )
                      else:
                          uv0 = uhalf[0].rearrange(
                              "p (gh k j) -> p gh k j", gh=2, k=K)
                          uv1 = uhalf[1].rearrange(
                              "p (gh k j) -> p gh k j", gh=2, k=K)
                          nc.vector.tensor_mul(
                              cuv[:, 0:2], uv0, bcast(c4[:, 0:2], K, 1))
                          nc.vector.tensor_mul(
                              cuv[:, 2:4], uv1, bcast(c4[:, 2:4], K, 1))
                      for gl in range(F):
                          for cch in range(4):
                              sl = slice(cch * 512, cch * 512 + 512)
                              nc.tensor.matmul(
                                  s_psum[:, sl],
                                  lhsT=odiag,
                                  rhs=cu[:, gl * JK + cch * 512:
                                         gl * JK + cch * 512 + 512],
                                  start=(S == 0 and gl == 0),
                                  stop=(S == SG - 1 and gl == F - 1),
                                  skip_group_check=True,
                              )

                  for S in range(SG + LAG):
                      if S < SG:
                          state[S] = front(S)
                      if S >= LAG:
                          back(S - LAG, state.pop(S - LAG))
                  v_rep = finish_iteration(s_psum, last=(it == ITERS - 1))

    nc.finalize()
    return nc


def _pack_inputs(x, W, n_cores, ntot=NTOT):
    """Shard over n, cast fp16, pre-transpose to the on-chip layouts."""
    nl = ntot // n_cores
    g = nl // 4
    sg = g // F
    in_maps = []
    for c in range(n_cores):
        wl = W[c * nl:(c + 1) * nl]                       # (nl, J, D, K)
        # supertile layout: [S, (n4 d), (gl, k, j)]
        wp = wl.reshape(sg, F, 4, J, DD, K)               # S gl n4 j d k
        wp = wp.transpose(0, 2, 4, 1, 5, 3)               # S n4 d gl k j
        wp = np.ascontiguousarray(
            wp.reshape(sg, 128, F * K * J).astype(np.float16)
        )
        xl = x[:, c * nl:(c + 1) * nl, :]                 # (B, nl, D)
        xg = xl.transpose(1, 2, 0).reshape(g, 4, DD, B).astype(np.float16)
        xt = np.ascontiguousarray(
            xg.reshape(g, 128, B).transpose(1, 0, 2)      # (128, g, b)
            .reshape(128, g * B)
        ) / np.float16(J)                                 # fold s1's 1/J scale
        xt = xt.astype(np.float16)
        xb = np.zeros((g, 128, 128), np.float16)
        for ns in range(4):
            xb[:, ns * 32:(ns + 1) * 32, ns * 32:(ns + 1) * 32] = xg[:, ns]
        xb = np.ascontiguousarray(
            xb.transpose(1, 0, 2).reshape(128, g * 128)
        )
        od = np.tile(np.eye(32, dtype=np.float16), (4, 1))
        in_maps.append({"w": wp, "xt": xt, "xb": xb, "od": od,
                        "id": np.eye(128, dtype=np.float16),
                        "idf": np.eye(128, dtype=np.float32)})
    return in_maps


def kernel(x, W):
    from concourse.bass_utils import run_bass_kernel_spmd

    x = np.asarray(x, dtype=np.float32)
    W = np.asarray(W, dtype=np.float32)
    g_res = int(os.environ.get("CAPS_G_RES", "12"))
    key = (NL, g_res, CORES)
    if key not in _CACHED:
        _CACHED[key] = _build_nc(NL, g_res, CORES)
    nc = _CACHED[key]
    in_maps = _pack_inputs(x, W, CORES)
    res = run_bass_kernel_spmd(nc, in_maps, list(range(CORES)))
    s = np.zeros((B, JK), np.float32)
    for c in range(CORES):
        s += np.asarray(res.results[c]["v"], dtype=np.float32)
    s = s.reshape(B, K, J).transpose(0, 2, 1)             # (k j) -> (B, J, K)
    s = np.ascontiguousarray(s)
    s2 = np.sum(s * s, axis=-1, keepdims=True)
    v = s2 / (1.0 + s2) / np.sqrt(s2 + 1e-8) * s
    return v.astype(np.float32)
